# revision 4
# baseline (speedup 1.0000x reference)
"""LSTM regression kernel for 8 Trainium2 NeuronCores.

Model (reference): B=2048, IN=2048, H=1024, T=15 steps, x constant across
steps. Data-parallel over batch: each of the 8 cores handles 256 batch rows.

Device strategy (per core, batch BL=256):
 - Everything kept "transposed": state hT/cT stored as [H, BL] with H on
   partitions (8 chunks of 128), so no per-step transposes are needed.
 - gatesT[4H, BL] = W_hh @ hT accumulated in PSUM over 8 K-chunks, plus one
   extra identity-weight matmul that adds the precomputed xgT tile (this
   replaces a per-tile DVE add of the input-gate contribution).
 - xgT[4H, BL] = W_ihAug @ xAugT computed once at start; biases b_ih+b_hh are
   folded in host-side by augmenting x with a ones-row and W_ih with a bias
   row (padded to a whole 128-row chunk).
 - Activations (sigmoid/tanh) on ScalarE directly from PSUM; cell update on
   VectorE per 128-row h-chunk so it pipelines with the matmuls.
 - Matmul inputs in bf16 (fp32 PSUM accumulate); h kept in fp32 for output
   and re-cast to bf16 each step for the next matmul.
"""

import os
import numpy as np
import ml_dtypes

try:
    import concourse.bass as bass
except ImportError:  # pragma: no cover
    import sys
    sys.path.insert(0, "/opt/trn_rl_repo")
    import concourse.bass as bass
from concourse import bacc
import concourse.mybir as mybir
import concourse.tile as tile
from concourse.bass_utils import run_bass_kernel_spmd
from concourse.masks import make_identity

F32 = mybir.dt.float32
BF16 = mybir.dt.bfloat16
AF = mybir.ActivationFunctionType

T = 15
B, IN, H = 2048, 2048, 1024
NCORES = 8
BL = B // NCORES            # 256 batch rows per core
G4 = 4 * H                  # 4096 gate rows
NM = G4 // 128              # 32 gate m-tiles
NKH = H // 128              # 8 hidden K-chunks
INA = IN + 128              # x augmented with ones row, padded to chunk
NKX = INA // 128            # 17 input K-chunks
INIT = 0.01

LAST_EXEC_NS = None
LAST_RESULTS = None

_cached_nc = None


def _build():
    nc = bacc.Bacc(None, target_bir_lowering=False)
    wih_hi = nc.dram_tensor("wih_hi", [INA, G4], BF16, kind="ExternalInput")
    wih_lo = nc.dram_tensor("wih_lo", [INA, G4], BF16, kind="ExternalInput")
    whh = nc.dram_tensor("whh", [H, G4], BF16, kind="ExternalInput")
    xt_hi = nc.dram_tensor("xt_hi", [INA, BL], BF16, kind="ExternalInput")
    xt_lo = nc.dram_tensor("xt_lo", [INA, BL], BF16, kind="ExternalInput")
    hs = nc.dram_tensor("hs", [T, 128, NKH * BL], F32, kind="ExternalOutput")

    with tile.TileContext(nc) as tc:
        with (
            tc.tile_pool(name="const", bufs=1) as constp,
            tc.tile_pool(name="wihp", bufs=4) as wihp,
            tc.tile_pool(name="state", bufs=2) as statep,
            tc.tile_pool(name="gates", bufs=3) as gatesp,
            tc.tile_pool(name="psum", bufs=8, space="PSUM") as psump,
        ):
            whh_sb = constp.tile([128, NKH * G4], BF16, tag="whh")
            xg_hi = constp.tile([128, NM * BL], BF16, tag="xghi")
            xg_lo = constp.tile([128, NM * BL], BF16, tag="xglo")
            xth_sb = constp.tile([128, NKX * BL], BF16, tag="xth")
            xtl_sb = constp.tile([128, NKX * BL], BF16, tag="xtl")
            ident = constp.tile([128, 128], BF16, tag="ident")
            make_identity(nc, ident[:, :])

            whh_r = whh[:, :].rearrange("(kc p) m -> kc p m", p=128)
            for kc in range(NKH):
                nc.sync.dma_start(whh_sb[:, kc * G4:(kc + 1) * G4], whh_r[kc])
            xth_r = xt_hi[:, :].rearrange("(kc p) b -> kc p b", p=128)
            xtl_r = xt_lo[:, :].rearrange("(kc p) b -> kc p b", p=128)
            for kc in range(NKX):
                nc.sync.dma_start(xth_sb[:, kc * BL:(kc + 1) * BL], xth_r[kc])
                nc.sync.dma_start(xtl_sb[:, kc * BL:(kc + 1) * BL], xtl_r[kc])

            # ---- xg phase: 4 sweeps, each producing ALL 4 gates for an
            # hc-pair (so recurrent step 0 for hc 0..1 can start after the
            # first sweep and overlap the rest of the xg phase) ----
            for sweep in range(4):
                pstiles = [psump.tile([128, BL], F32, tag="ps", name=f"psxg{i}") for i in range(8)]
                for kc in range(NKX):
                    wth = wihp.tile([128, 1024], BF16, tag="wihh", name="wth")
                    wtl = wihp.tile([128, 1024], BF16, tag="wihl", name="wtl")
                    src_h = wih_hi[kc * 128:(kc + 1) * 128, :].rearrange(
                        "p (g t c) -> p g t c", g=4, t=4
                    )[:, :, sweep, :]
                    src_l = wih_lo[kc * 128:(kc + 1) * 128, :].rearrange(
                        "p (g t c) -> p g t c", g=4, t=4
                    )[:, :, sweep, :]
                    nc.sync.dma_start(wth[:, :], src_h)
                    nc.sync.dma_start(wtl[:, :], src_l)
                    for ml in range(8):
                        for pi, (wt_, xt_) in enumerate(
                            [(wth, xth_sb), (wth, xtl_sb), (wtl, xth_sb)]
                        ):
                            nc.tensor.matmul(
                                pstiles[ml][:, :],
                                wt_[:, ml * 128:(ml + 1) * 128],
                                xt_[:, kc * BL:(kc + 1) * BL],
                                start=(kc == 0 and pi == 0),
                                stop=(kc == NKX - 1 and pi == 2),
                            )
                for ml in range(8):
                    g_, j_ = ml // 2, ml % 2
                    m = g_ * 8 + sweep * 2 + j_
                    nc.scalar.copy(xg_hi[:, m * BL:(m + 1) * BL], pstiles[ml][:, :])
                    nc.vector.tensor_sub(
                        xg_lo[:, m * BL:(m + 1) * BL],
                        pstiles[ml][:, :],
                        xg_hi[:, m * BL:(m + 1) * BL],
                    )

            # ---- recurrent steps ----
            h_prev = statep.tile([128, NKH * BL], BF16, tag="hbf")
            c_prev = statep.tile([128, NKH * BL], F32, tag="c")
            nc.any.memset(h_prev[:, :], INIT)
            nc.any.memset(c_prev[:, :], INIT)

            for t in range(T):
                h_bf = statep.tile([128, NKH * BL], BF16, tag="hbf")
                h_f32 = statep.tile([128, NKH * BL], F32, tag="hf")
                c_new = statep.tile([128, NKH * BL], F32, tag="c")
                for hc in range(NKH):
                    gt = []
                    for gi in range(4):
                        m = gi * NKH + hc
                        ps = psump.tile([128, BL], F32, tag="ps", name="psrec")
                        for kc in range(NKH):
                            nc.tensor.matmul(
                                ps[:, :],
                                whh_sb[:, kc * G4 + m * 128: kc * G4 + (m + 1) * 128],
                                h_prev[:, kc * BL:(kc + 1) * BL],
                                start=(kc == 0),
                                stop=False,
                            )
                        nc.tensor.matmul(
                            ps[:, :],
                            ident[:, :],
                            xg_hi[:, m * BL:(m + 1) * BL],
                            start=False,
                            stop=False,
                        )
                        nc.tensor.matmul(
                            ps[:, :],
                            ident[:, :],
                            xg_lo[:, m * BL:(m + 1) * BL],
                            start=False,
                            stop=True,
                        )
                        g = gatesp.tile([128, BL], F32, tag=f"g{gi}", name=f"gate{gi}")
                        fn = AF.Tanh if gi == 2 else AF.Sigmoid
                        nc.scalar.activation(g[:, :], ps[:, :], fn)
                        gt.append(g)
                    sl = slice(hc * BL, (hc + 1) * BL)
                    t0 = gatesp.tile([128, BL], F32, tag="t0")
                    t1 = gatesp.tile([128, BL], F32, tag="t1")
                    th = gatesp.tile([128, BL], F32, tag="th")
                    nc.vector.tensor_mul(t0[:, :], gt[0][:, :], gt[2][:, :])
                    nc.vector.tensor_mul(t1[:, :], gt[1][:, :], c_prev[:, sl])
                    nc.vector.tensor_add(c_new[:, sl], t0[:, :], t1[:, :])
                    nc.scalar.activation(th[:, :], c_new[:, sl], AF.Tanh)
                    nc.vector.tensor_mul(h_f32[:, sl], gt[3][:, :], th[:, :])
                    nc.vector.tensor_copy(h_bf[:, sl], h_f32[:, sl])
                nc.sync.dma_start(hs[t], h_f32[:, :])
                h_prev, c_prev = h_bf, c_new

    nc.compile()
    return nc


def timeline_ns():
    from concourse.timeline_sim import TimelineSim
    nc = _get_nc()
    ts = TimelineSim(nc)
    ts.simulate()
    return ts.time


def _get_nc():
    global _cached_nc
    if _cached_nc is None:
        _cached_nc = _build()
    return _cached_nc


def kernel(x, W_ih, W_hh, b_ih, b_hh):
    global LAST_EXEC_NS, LAST_RESULTS
    nc = _get_nc()
    bf = ml_dtypes.bfloat16
    x = np.asarray(x, np.float32)
    W_ih = np.asarray(W_ih, np.float32)
    W_hh = np.asarray(W_hh, np.float32)
    b_ih = np.asarray(b_ih, np.float32)
    b_hh = np.asarray(b_hh, np.float32)

    def hilo(a):
        hi = a.astype(bf)
        lo = (a - hi.astype(np.float32)).astype(bf)
        return hi, lo

    waug = np.zeros((INA, G4), np.float32)
    waug[:IN] = W_ih.T
    waug[IN] = b_ih + b_hh
    waug_hi, waug_lo = hilo(waug)
    whh_bf = np.ascontiguousarray(W_hh.T).astype(bf)

    in_maps = []
    for c in range(NCORES):
        xa = np.zeros((INA, BL), np.float32)
        xa[:IN] = x[c * BL:(c + 1) * BL].T
        xa[IN] = 1.0
        xa_hi, xa_lo = hilo(xa)
        in_maps.append({
            "wih_hi": waug_hi, "wih_lo": waug_lo, "whh": whh_bf,
            "xt_hi": xa_hi, "xt_lo": xa_lo,
        })

    trace = os.environ.get("LSTM_TRACE") == "1"
    res = run_bass_kernel_spmd(
        nc, in_maps, core_ids=list(range(NCORES)), trace=trace
    )
    LAST_EXEC_NS = res.exec_time_ns
    LAST_RESULTS = res

    out = np.empty((T, B, H), np.float32)
    for c in range(NCORES):
        a = res.results[c]["hs"].reshape(T, 128, NKH, BL)
        out[:, c * BL:(c + 1) * BL, :] = (
            a.transpose(0, 3, 2, 1).reshape(T, BL, H)
        )
    return out
